# revision 22
# baseline (speedup 1.0000x reference)
"""GAT (graph attention) layer on 8 TRN2 NeuronCores — Bass/Tile kernel.

Sharding: destination-node dim i is split across the 8 cores (256 rows
each).  Wh and params are replicated; softmax is over j within a row so
no collective is needed.

Math rewrite (per core, rows i in its shard):
  exp(lrelu(e_i+e_j)) = max(exp(s), exp(alpha*s)); dividing the softmax
  weights by exp(e_i)*exp(e_j) (cancels in the normalization):
    weight[i,j,h]  propto  v_j * min(max(r_i * w_j, 1), HUGE*adj)
  with v_j = exp(e_j), w_j = exp((alpha-1) e_j), r_i = exp((alpha-1) e_i).
  No transcendentals on the [N,NL,H] logits tensor: exps only on [N,H]
  vectors, computed once in a batched pre-phase.  Per j-tile:
    PE:  Y2[j,(h,i)] = r_i * w_j        (K=8 matmul, lhsT = wT slice)
    ACT: T0 = bf16(Y2)                  (PSUM evacuation, halves)
    DVE: T1 = max(T0, 1.0)              (tensor_scalar, 4x mode)
    DVE: G  = min(T1, adjHUGE)          (tensor_tensor, 2x mode)
    PE:  agg[d,(h,i)] += VWh^T @ G      (stationary VWh, 8 mm/tile)
  v folds into the agg lhsT (VWh); its ones-row gives the denominator.
  agg is produced transposed ([65,h,i]) and transposed back once at the
  end (16 PE transposes) before the ELU finalize.

Host-side prep: hT pre-transposed bf16 blocks, W_aug = [W | W@a_j_h |
W@a_i_h] folded in numpy, adjT scaled by 1e30 as the mask-min operand.
"""

import dataclasses
import sys

import numpy as np

sys.path.insert(0, "/opt/trn_rl_repo")

N = 2048
F_IN = 768
F_OUT = 64
H = 8
ALPHA = 0.2
NCORES = 8
NL = N // NCORES          # 256 local rows per core
KT = F_IN // 128          # 6 k-tiles
NT = N // 128              # 16 j tiles
FH = F_OUT * H            # 512
FW = FH + 2 * H           # 528: [W | wa_j | wa_i] folded rhs
HUGE = 1e30

_CACHE = {}


def _build():
    import concourse.bacc as bacc
    import concourse.mybir as mybir
    from concourse.tile import TileContext

    f32 = mybir.dt.float32
    bf16 = mybir.dt.bfloat16
    AF = mybir.ActivationFunctionType
    OP = mybir.AluOpType

    nc = bacc.Bacc("TRN2", target_bir_lowering=False, debug=False,
                   num_devices=NCORES)

    # hT blocks: [nt, k, 128, 128]; block (t,k) = h[t-tile, k-tile].T
    hT_d = nc.declare_dram_parameter("hT", [NT * KT * 128, 128], bf16,
                                     isOutput=False)
    hlT_d = nc.declare_dram_parameter("hlT", [KT * 128, NL], bf16,
                                      isOutput=False)
    Waug_d = nc.declare_dram_parameter("Waug", [KT * 128, FW], bf16,
                                       isOutput=False)
    adjH_d = nc.declare_dram_parameter("adjH", [N, NL], bf16, isOutput=False)
    out_d = nc.declare_dram_parameter("out", [NL, FH], f32, isOutput=True)

    with TileContext(nc) as tc:
        with tc.tile_pool(name="persist", bufs=1) as pp:
            ident = pp.tile([128, 128], f32)
            ident_b = pp.tile([128, 128], bf16)
            Waug_sb = pp.tile([128, KT, FW], bf16)
            hT_sb = pp.tile([128, NT, KT, 128], bf16)
            hlT_sb = pp.tile([128, KT, 2, 128], bf16)
            adjH_sb = pp.tile([128, NT, NL], bf16)
            rhsY2 = pp.tile([H, H * NL], bf16)
            vT8 = pp.tile([H, N], bf16)
            wT_all = pp.tile([H, N], bf16)
            v_all = pp.tile([128, NT, H], bf16)
            sb_agg = pp.tile([65, H, NL], f32)
            r_sb = pp.tile([128, NT], f32)
            dn_sb = pp.tile([128, NT], f32)
            hp_sb = pp.tile([128, 2, FH], f32)
            mn_sb = pp.tile([128, 2, FH], f32)
            em_sb = pp.tile([128, 2, FH], f32)
            out_sb = pp.tile([128, 2, FH], f32)
            zs_row = pp.tile([1, 512], f32)

            # ---------- pre-phase ----------
            with tc.tile_pool(name="pre", bufs=2) as sp, \
                 tc.tile_pool(name="preps", bufs=2, space="PSUM") as ps:

                io_t = sp.tile([128, 128], mybir.dt.int32, tag="iota")
                nc.gpsimd.iota(io_t[:], pattern=[[-1, 128]], base=0,
                               channel_multiplier=1)
                nc.vector.tensor_scalar(ident[:], io_t[:], 0, None,
                                        OP.is_equal)
                nc.vector.tensor_copy(ident_b[:], ident[:])
                nc.vector.memset(zs_row[:], 0.0)
                nc.vector.memset(rhsY2[:], 0.0)

                for k in range(KT):
                    nc.sync.dma_start(out=Waug_sb[:, k, :],
                                      in_=Waug_d[k * 128:(k + 1) * 128, :])
                for k in range(KT):
                    nc.sync.dma_start(
                        out=hlT_sb[:, k, :, :].rearrange("p a b -> p (a b)"),
                        in_=hlT_d[k * 128:(k + 1) * 128, :])
                for jt in range(NT):
                    nc.sync.dma_start(
                        out=hT_sb[:, jt, :, :].rearrange("p a b -> p (a b)"),
                        in_=hT_d[jt * KT * 128:(jt + 1) * KT * 128,
                                 :].rearrange("(c k) j -> c (k j)", k=KT))
                for jt in range(NT):
                    nc.sync.dma_start(out=adjH_sb[:, jt, :],
                                      in_=adjH_d[jt * 128:(jt + 1) * 128, :])

                # e_i -> r_i = exp((a-1)e_i) -> rhsY2 diagonal blocks
                for lt in range(2):
                    ps_ei = ps.tile([128, H], f32, tag="ei")
                    for k in range(KT):
                        nc.tensor.matmul(ps_ei[:], hlT_sb[:, k, lt, :],
                                         Waug_sb[:, k, FH + H:FW],
                                         start=(k == 0), stop=(k == KT - 1))
                    r32 = sp.tile([128, H], f32, tag="r32")
                    nc.scalar.activation(r32[:], ps_ei[:], AF.Exp,
                                         scale=ALPHA - 1.0)
                    ps_rT = ps.tile([H, 128], f32, tag="rT")
                    nc.tensor.transpose(ps_rT[:], r32[:], ident[:])
                    rT = sp.tile([H, 128], bf16, tag="rTb")
                    nc.vector.tensor_copy(rT[:], ps_rT[:])
                    for hh in range(H):
                        nc.sync.dma_start(
                            out=rhsY2[hh:hh + 1,
                                      hh * NL + lt * 128:
                                      hh * NL + (lt + 1) * 128],
                            in_=rT[hh:hh + 1, :])

                # batched e_j: ejT[h, j] via stationary Wa_j (8-col lhsT);
                # v/w exps straight into [8, N] rows.
                for cc in range(4):
                    c0 = cc * 512
                    ps_ej = ps.tile([H, 512], f32, tag="ej")
                    for k in range(KT):
                        nc.tensor.matmul(
                            ps_ej[:],
                            Waug_sb[:, k, FH:FH + H],
                            hT_sb[:, cc * 4:(cc + 1) * 4, k, :],
                            start=(k == 0), stop=(k == KT - 1))
                    nc.scalar.activation(vT8[:, c0:c0 + 512], ps_ej[:],
                                         AF.Exp)
                    nc.scalar.activation(wT_all[:, c0:c0 + 512], ps_ej[:],
                                         AF.Exp, scale=ALPHA - 1.0)
                # v in per-partition layout via PE transposes (pre-phase
                # PSUM is otherwise idle)
                for jt in range(NT):
                    ps_v = ps.tile([128, H], bf16, tag="vt")
                    nc.tensor.transpose(ps_v[:],
                                        vT8[:, jt * 128:(jt + 1) * 128],
                                        ident_b[0:H, 0:H])
                    nc.vector.tensor_copy(v_all[:, jt, :], ps_v[:])

            # ---------- main loop over j-tiles ----------
            with tc.tile_pool(name="ev", bufs=2) as ev, \
                 tc.tile_pool(name="gv", bufs=2) as gv, \
                 tc.tile_pool(name="psm", bufs=1, space="PSUM") as psm, \
                 tc.tile_pool(name="psy", bufs=1, space="PSUM") as pyp:

                # agg (transposed): [65, h, i]; row 64 = denominator
                ps_agg = psm.tile([65, H, NL], f32, tag="agg")
                agg_flat = ps_agg[:].rearrange("p g d -> p (g d)")
                for off in range(0, H * NL, 512):
                    nc.tensor.matmul(agg_flat[:, off:off + 512],
                                     zs_row[0:1, 0:65],
                                     zs_row[0:1, 0:512],
                                     start=True, stop=False,
                                     skip_group_check=True)

                HW2 = H // 2 * NL            # 1024 cols per half
                prev = None                   # (G, VWh) of previous tile

                def agg_mm(Gt, VWht, jt):
                    for hh in range(H):
                        nc.tensor.matmul(
                            ps_agg[:, hh, :],
                            VWht[:, hh, :],
                            Gt[:, hh * NL:(hh + 1) * NL],
                            start=False, stop=(jt == NT - 1),
                            skip_group_check=True)

                for jt in range(NT):
                    G = gv.tile([128, H * NL], bf16, tag="g")
                    base = adjH_sb[:, jt, :]
                    for hf in range(2):
                        f0 = hf * HW2
                        ps_y = pyp.tile([128, HW2], f32, tag="y")
                        for q in range(2):
                            nc.tensor.matmul(
                                ps_y[:, q * 512:(q + 1) * 512],
                                wT_all[:, jt * 128:(jt + 1) * 128],
                                rhsY2[:, f0 + q * 512:f0 + (q + 1) * 512],
                                start=True, stop=True)
                        if hf == 0 and prev is not None:
                            agg_mm(*prev, jt - 1)
                        T0 = ev.tile([128, HW2], bf16, tag="t0")
                        nc.scalar.copy(out=T0[:], in_=ps_y[:])
                        T1 = ev.tile([128, HW2], bf16, tag="t1")
                        nc.vector.tensor_scalar(T1[:], T0[:], 1.0, None,
                                                OP.max)
                        rep = dataclasses.replace(
                            base, ap=[list(base.ap[0]), [0, H // 2],
                                      list(base.ap[1])])
                        nc.vector.tensor_tensor(
                            G[:, f0:f0 + HW2].rearrange(
                                "p (h i) -> p h i", h=H // 2),
                            T1[:].rearrange("p (h i) -> p h i", h=H // 2),
                            rep, OP.min)

                    # Wh for this tile; VWh = v * Wh (+ v into ones row)
                    ps_w = psm.tile([128, FH], f32, tag="w")
                    for k in range(KT):
                        nc.tensor.matmul(ps_w[:],
                                         hT_sb[:, jt, k, :],
                                         Waug_sb[:, k, 0:FH],
                                         start=(k == 0), stop=(k == KT - 1))
                    VWh = gv.tile([128, H, F_OUT + 1], bf16, tag="vwh")
                    v_sl = v_all[:, jt, :]
                    v_rep = dataclasses.replace(
                        v_sl, ap=[list(v_sl.ap[0]), list(v_sl.ap[1]),
                                  [0, F_OUT]])
                    nc.vector.tensor_tensor(
                        VWh[:, :, 0:F_OUT],
                        ps_w[:].rearrange("p (h d) -> p h d", h=H),
                        v_rep, OP.mult)
                    nc.vector.tensor_copy(VWh[:, :, F_OUT], v_sl)

                    prev = (G, VWh)

                agg_mm(*prev, NT - 1)

                # ---------- finalize ----------
                # evac transposed agg, transpose back to [i, (g, 65)];
                # alternate ACT/DVE for the per-group evacuations.
                nc.scalar.copy(out=sb_agg[:], in_=ps_agg[:])
                for hh in range(H):
                    for ih in range(2):
                        g = hh * 2 + ih
                        ps_t = pyp.tile([128, 65], f32, tag="ft")
                        nc.tensor.transpose(
                            ps_t[:],
                            sb_agg[:, hh, ih * 128:(ih + 1) * 128],
                            ident[0:65, 0:65])
                        dst = hp_sb[:, ih, hh * F_OUT:(hh + 1) * F_OUT]
                        if g % 2 == 0:
                            nc.vector.tensor_copy(dst, ps_t[:, 0:F_OUT])
                        else:
                            nc.scalar.copy(out=dst, in_=ps_t[:, 0:F_OUT])
                        nc.vector.tensor_copy(dn_sb[:, g:g + 1],
                                              ps_t[:, F_OUT:F_OUT + 1])
                nc.vector.reciprocal(r_sb[:], dn_sb[:])
                # hp *= recip(den): one pass, r broadcast over d (stride-0)
                r_v = r_sb[:]
                r_rep = dataclasses.replace(
                    r_v, ap=[list(r_v.ap[0]), [1, 2], [2, 8], [0, F_OUT]])
                nc.vector.tensor_tensor(
                    hp_sb[:].rearrange("p a (h d) -> p a h d", h=H),
                    hp_sb[:].rearrange("p a (h d) -> p a h d", h=H),
                    r_rep, OP.mult)
                nc.vector.tensor_scalar(mn_sb[:], hp_sb[:], 0.0, None, OP.min)
                nc.scalar.activation(em_sb[:], mn_sb[:], AF.Exp)
                nc.vector.scalar_tensor_tensor(out_sb[:], em_sb[:], -1.0,
                                               hp_sb[:], OP.add, OP.max)
                for ih in range(2):
                    nc.sync.dma_start(out=out_d[ih * 128:(ih + 1) * 128, :],
                                      in_=out_sb[:, ih, :])

    nc.compile()
    return nc


def kernel(h, adj, W, a):
    import ml_dtypes
    from concourse.bass_utils import run_bass_kernel_spmd

    if "nc" not in _CACHE:
        _CACHE["nc"] = _build()
    nc = _CACHE["nc"]

    bf16 = ml_dtypes.bfloat16
    h = np.ascontiguousarray(h, dtype=np.float32)
    adj = np.ascontiguousarray(adj, dtype=np.float32)
    W = np.ascontiguousarray(W, dtype=np.float32)
    a = np.ascontiguousarray(a, dtype=np.float32)

    a_i = a[0, :, :F_OUT]                    # [H, D]
    a_j = a[0, :, F_OUT:]                    # [H, D]
    Wr = W.reshape(F_IN, H, F_OUT)
    Wa_j = np.einsum('khd,hd->kh', Wr, a_j)  # [F_IN, H]
    Wa_i = np.einsum('khd,hd->kh', Wr, a_i)
    Waug = np.concatenate([W, Wa_j, Wa_i], axis=1).astype(bf16)  # [768, 528]

    # hT blocks [nt, k, 128, 128]: block = h[t-tile, k-tile].T
    # [t, c, k, j]: row (t*128+c) holds h[t*128+j, k*128+c] for all (k, j);
    # the kernel reads it back k-major via a rearranged DMA source AP.
    hT = np.ascontiguousarray(
        h.reshape(NT, 128, KT, 128).transpose(0, 3, 2, 1)).astype(bf16)
    hT = hT.reshape(NT * KT * 128, 128)

    in_maps = []
    for c in range(NCORES):
        sl = slice(c * NL, (c + 1) * NL)
        hl = h[sl]                            # [256, 768]
        hlT = np.ascontiguousarray(
            hl.reshape(2, 128, KT, 128).transpose(2, 3, 0, 1)).astype(bf16)
        hlT = hlT.reshape(KT * 128, NL)       # [k*128, (lt,128)]
        adjH = ((adj[sl].T > 0) * np.float32(HUGE)).astype(bf16)
        in_maps.append({
            "hT": hT,
            "hlT": hlT,
            "Waug": Waug,
            "adjH": np.ascontiguousarray(adjH),
        })
    res = run_bass_kernel_spmd(nc, in_maps, list(range(NCORES)),
                               trace=bool(_CACHE.get("trace")))
    _CACHE["last"] = res
    return np.concatenate([res.results[c]["out"] for c in range(NCORES)],
                          axis=0)
